# revision 4
# baseline (speedup 1.0000x reference)
"""Bounding-box kernel for Trainium2 (Bass/Tile), 8-core SPMD.

Problem: mask [128, 1, 512, 512] f32 -> bbox [128, 4] int32
  (y_min, x_min, y_max, x_max) of the region where mask >= 0.5,
  with (0, 0, H, W) when a row/col has no hit.

Strategy (per core, 16 images):
  - DMA each image [512, 512] as one [128, 4, 512] tile (H split into 4
    partition blocks).
  - Column extents: ACT computes t = Relu(x*2^24 + (1 - 2^23)) which is
    exactly 0 iff x < 0.5 and >= 1 otherwise (exact in f32 for the
    threshold boundary), output bf16. One-hot [128, 16] lhsT matmuls
    accumulate per-image column "counts" for all 16 images into a single
    PSUM [16, 512] tile (partition = image).
  - Row extents: Pool reduce_max over W per block -> [128, 64] (col =
    b*16 + i), compare >= 0.5, PE-transpose the four [128, 16] slices
    into PSUM [16, 512] (partition = image, free = H).
  - First/last hit index via fused DVE tensor_tensor_reduce:
      min(hit * (f - 512)) + 512  -> first hit  (512 if none)
      max(hit * (f + 1))          -> last hit+1 (0 if none)
    plus a no-hit fixup, f32 -> int32 copy, tiny DMA out.
"""

import numpy as np
import ml_dtypes
from contextlib import ExitStack

import concourse.bass as bass
import concourse.bacc as bacc
import concourse.tile as tile
import concourse.mybir as mybir
from concourse.bass_utils import run_bass_kernel_spmd

N_CORES = 8
N, H, W = 128, 512, 512
NPC = N // N_CORES          # images per core = 16
P = 128                     # SBUF partitions
NBLK = H // P               # 4 row blocks per image
F32 = mybir.dt.float32
BF16 = mybir.dt.bfloat16
I32 = mybir.dt.int32

# Relu(x * 2^25 - (2^24 - 1)) == 0 iff x < 0.5, >= 1 iff x >= 0.5, exact
# for EVERY f32 x: x*2^25 is exact (power-of-2 scale); for x < 0.5,
# x*2^25 <= 2^24 - 1 so the true sum is <= 0 (rounding is monotone, 0 is
# representable); for x >= 0.5 the true sum is >= 1 and rounds to >= 1.
ACT_SCALE = float(2**25)
ACT_BIAS = float(1 - 2**24)

TRACE = False               # test.py sets True to capture a HW profile
LAST_RESULTS = None         # BassKernelResults of the last run

_compiled = None


def _build_nc():
    nc = bacc.Bacc(
        "TRN2", target_bir_lowering=False, debug=False, num_devices=N_CORES
    )
    mask_d = nc.dram_tensor("mask", [NPC * H, W], F32, kind="ExternalInput").ap()
    oneh_d = nc.dram_tensor("onehot", [P, NPC * NPC], BF16, kind="ExternalInput").ap()
    ident_d = nc.dram_tensor("ident", [P, P], F32, kind="ExternalInput").ap()
    xlo_d = nc.dram_tensor("xlo_const", [NPC, W], F32, kind="ExternalInput").ap()
    xhi_d = nc.dram_tensor("xhi_const", [NPC, W], F32, kind="ExternalInput").ap()
    ylo_d = nc.dram_tensor("ylo_const", [NPC, H], F32, kind="ExternalInput").ap()
    yhi_d = nc.dram_tensor("yhi_const", [NPC, H], F32, kind="ExternalInput").ap()
    bbox_d = nc.dram_tensor("bbox", [NPC, 4], I32, kind="ExternalOutput").ap()

    with tile.TileContext(nc) as tc, ExitStack() as ctx:
        consts = ctx.enter_context(tc.tile_pool(name="consts", bufs=1))
        xpool = ctx.enter_context(tc.tile_pool(name="x", bufs=8))
        hpool = ctx.enter_context(tc.tile_pool(name="h", bufs=16))
        hspool = ctx.enter_context(tc.tile_pool(name="hs", bufs=8))
        lastpool = ctx.enter_context(tc.tile_pool(name="last", bufs=2))
        small = ctx.enter_context(tc.tile_pool(name="small", bufs=1))
        scratch = ctx.enter_context(tc.tile_pool(name="scratch", bufs=2))
        psum = ctx.enter_context(tc.tile_pool(name="psum", bufs=1, space="PSUM"))

        oneh = consts.tile([P, NPC * NPC], BF16)
        nc.sync.dma_start(out=oneh[:], in_=oneh_d)
        ident = consts.tile([P, P], F32)
        nc.sync.dma_start(out=ident[:], in_=ident_d)
        xlo_c = consts.tile([NPC, W], F32)
        nc.sync.dma_start(out=xlo_c[:], in_=xlo_d)
        xhi_c = consts.tile([NPC, W], F32)
        nc.sync.dma_start(out=xhi_c[:], in_=xhi_d)
        ylo_c = consts.tile([NPC, H], F32)
        nc.sync.dma_start(out=ylo_c[:], in_=ylo_d)
        yhi_c = consts.tile([NPC, H], F32)
        nc.sync.dma_start(out=yhi_c[:], in_=yhi_d)
        act_bias = consts.tile([P, 1], F32)
        nc.vector.memset(act_bias[:], ACT_BIAS)

        # H is loaded interleaved: partition p, sub-row j <-> h = 4p + j
        # (4 contiguous HBM rows per partition -> 8KB DMA descriptors).
        # col j*16 + i of rowmax holds the row max of image i, sub-row j.
        rowmax = small.tile([P, NBLK * NPC], F32)
        rowmax_v = rowmax.rearrange("p (b i) -> p b i", b=NBLK)
        cnt_ps = psum.tile([NPC, W], F32)    # per-image column counts
        trow_ps = psum.tile([NPC, H], F32)   # per-image row hits (transposed)

        for i in range(NPC - 1):
            x = xpool.tile([P, NBLK, W], F32, tag="x")
            # alternate images between the two HWDGE queues (SP / Act) so
            # both hardware DMA paths pull from HBM concurrently
            dma_eng = nc.sync if i % 2 == 0 else nc.scalar
            dma_eng.dma_start(
                out=x[:],
                in_=mask_d[i * H:(i + 1) * H, :].rearrange("(p b) w -> p b w", p=P),
            )
            h = hpool.tile([P, NBLK, W], BF16, tag="h")
            nc.scalar.activation(
                h[:], x[:], mybir.ActivationFunctionType.Relu,
                bias=act_bias[:], scale=ACT_SCALE,
            )
            nc.vector.tensor_reduce(
                out=rowmax_v[:, :, i], in_=x[:],
                axis=mybir.AxisListType.X, op=mybir.AluOpType.max,
            )
            lhsT = oneh[:, i * NPC:(i + 1) * NPC]
            if i < NPC - 2:
                # pre-sum block pairs on the otherwise-idle gpsimd engine:
                # halves the PE matmul count (hit-mass stays 0 iff no hit)
                hs = hspool.tile([P, 2, W], BF16)
                nc.gpsimd.tensor_add(hs[:, 0, :], h[:, 0, :], h[:, 1, :])
                nc.gpsimd.tensor_add(hs[:, 1, :], h[:, 2, :], h[:, 3, :])
                for s in range(2):
                    nc.tensor.matmul(
                        cnt_ps[:, :], lhsT, hs[:, s, :],
                        start=(i == 0 and s == 0), stop=False,
                    )
            else:
                # penultimate image: skip the gpsimd hop (shorter tail)
                for b in range(NBLK):
                    nc.tensor.matmul(
                        cnt_ps[:, :], lhsT, h[:, b, :],
                        start=False, stop=False,
                    )

        # last image: two half loads so its compute chain starts while the
        # second half is still in flight
        i = NPC - 1
        lhsT = oneh[:, i * NPC:(i + 1) * NPC]
        for u in range(2):
            x = lastpool.tile([P, 2, W], F32, tag="xh")
            nc.scalar.dma_start(
                out=x[:],
                in_=mask_d[i * H:(i + 1) * H, :]
                .rearrange("(p b) w -> p b w", p=P)[:, 2 * u:2 * u + 2, :],
            )
            h = lastpool.tile([P, 2, W], BF16, tag="hh")
            nc.scalar.activation(
                h[:], x[:], mybir.ActivationFunctionType.Relu,
                bias=act_bias[:], scale=ACT_SCALE,
            )
            nc.vector.tensor_reduce(
                out=rowmax_v[:, 2 * u:2 * u + 2, i], in_=x[:],
                axis=mybir.AxisListType.X, op=mybir.AluOpType.max,
            )
            for b in range(2):
                nc.tensor.matmul(
                    cnt_ps[:, :], lhsT, h[:, b, :],
                    start=False, stop=(u == 1 and b == 1),
                )

        # rows: hit01 then transpose blocks into [16, 512]
        rowhit = small.tile([P, NBLK * NPC], F32)
        nc.vector.tensor_scalar(
            rowhit[:], rowmax[:], 0.5, None, mybir.AluOpType.is_ge
        )
        rowhit_v = rowhit.rearrange("p (b i) -> p b i", b=NBLK)
        for b in range(NBLK):
            nc.tensor.matmul(
                trow_ps[:, b * P:(b + 1) * P], rowhit_v[:, b, :], ident[:],
                is_transpose=True, start=True, stop=True,
            )

        # raw extents tile: col 0 = ylo, 1 = xlo, 2 = yhi, 3 = xhi
        # (lo values are lo-512 for hit, 0 for none; hi are hi or 0)
        raw = small.tile([NPC, 4], F32)

        # NOTE: tensor_tensor_reduce and scalar_tensor_tensor (fused DVE
        # ISA ops) both crash the exec unit on this runtime path; use
        # plain compare/mul + reduce.
        colhit = small.tile([NPC, W], F32)
        nc.vector.tensor_scalar(
            colhit[:], cnt_ps[:], 0.5, None, mybir.AluOpType.is_ge
        )

        def extents(hit_ap, lo_c, hi_c, lo_out, hi_out):
            prod = scratch.tile([NPC, W], F32, tag="prod")
            nc.vector.tensor_mul(prod[:], hit_ap, lo_c[:])
            nc.vector.tensor_reduce(
                out=lo_out, in_=prod[:],
                axis=mybir.AxisListType.X, op=mybir.AluOpType.min,
            )
            prod2 = scratch.tile([NPC, W], F32, tag="prod")
            nc.vector.tensor_mul(prod2[:], hit_ap, hi_c[:])
            nc.vector.tensor_reduce(
                out=hi_out, in_=prod2[:],
                axis=mybir.AxisListType.X, op=mybir.AluOpType.max,
            )

        extents(trow_ps[:], ylo_c, yhi_c, raw[:, 0:1], raw[:, 2:3])
        extents(colhit[:], xlo_c, xhi_c, raw[:, 1:2], raw[:, 3:4])

        # lo_final = (lo_raw + 512) * (1 - nohit); hi_final = hi_raw + 512*nohit
        # where nohit = (hi_raw == 0). bbox layout: (ymin, xmin, ymax, xmax);
        # both lo (and both hi) columns are adjacent, so fix up 2-wide.
        bbox_f = small.tile([NPC, 4], F32)
        m2 = small.tile([NPC, 2], F32)
        nc.vector.tensor_scalar(m2[:], raw[:, 2:4], 0.0, None, mybir.AluOpType.is_equal)
        t2 = small.tile([NPC, 2], F32)
        nc.vector.tensor_scalar_add(t2[:], raw[:, 0:2], float(H))
        v2 = small.tile([NPC, 2], F32)
        nc.vector.tensor_mul(v2[:], t2[:], m2[:])
        nc.vector.tensor_sub(bbox_f[:, 0:2], t2[:], v2[:])
        w2 = small.tile([NPC, 2], F32)
        nc.vector.tensor_scalar_mul(w2[:], m2[:], float(H))
        nc.vector.tensor_add(bbox_f[:, 2:4], raw[:, 2:4], w2[:])

        bbox_i = small.tile([NPC, 4], I32)
        nc.vector.tensor_copy(bbox_i[:], bbox_f[:])
        nc.sync.dma_start(out=bbox_d, in_=bbox_i[:])

    nc.compile()
    return nc


def _consts():
    oneh = np.zeros((P, NPC * NPC), dtype=ml_dtypes.bfloat16)
    for i in range(NPC):
        oneh[:, i * NPC + i] = 1.0
    ident = np.eye(P, dtype=np.float32)
    f = np.arange(W, dtype=np.float32)
    xlo = np.broadcast_to(f - W, (NPC, W)).copy()
    xhi = np.broadcast_to(f + 1, (NPC, W)).copy()
    # trow free index f maps to image row h = 4*(f % 128) + f // 128
    fi = np.arange(H)
    hperm = (4 * (fi % P) + fi // P).astype(np.float32)
    ylo = np.broadcast_to(hperm - H, (NPC, H)).copy()
    yhi = np.broadcast_to(hperm + 1, (NPC, H)).copy()
    return oneh, ident, xlo, xhi, ylo, yhi


def kernel(mask):
    global _compiled, LAST_RESULTS
    mask = np.ascontiguousarray(np.asarray(mask), dtype=np.float32)
    assert mask.shape == (N, 1, H, W), mask.shape
    if _compiled is None:
        _compiled = _build_nc()
    nc = _compiled
    oneh, ident, xlo, xhi, ylo, yhi = _consts()
    m = mask.reshape(N, H, W)
    in_maps = []
    for c in range(N_CORES):
        in_maps.append({
            "mask": np.ascontiguousarray(
                m[c * NPC:(c + 1) * NPC].reshape(NPC * H, W)
            ),
            "onehot": oneh,
            "ident": ident,
            "xlo_const": xlo,
            "xhi_const": xhi,
            "ylo_const": ylo,
            "yhi_const": yhi,
        })
    res = run_bass_kernel_spmd(nc, in_maps, list(range(N_CORES)), trace=TRACE)
    LAST_RESULTS = res
    out = np.concatenate([res.results[c]["bbox"] for c in range(N_CORES)], axis=0)
    return out.astype(np.int32, copy=False)

